# revision 1
# baseline (speedup 1.0000x reference)
"""Euclidean distance (cdist) kernel for Trainium2, 8 NeuronCores.

out[b, j] = || x[b, :] - weight[:, j] ||_2   for x [4096, 64], weight [64, 50000].

Sharding (per hint): K = 50000 split into 8 slabs of 6250, one per core
(tensor-parallel over prototypes); x replicated; no cross-core reduction.

Math: dist^2 = x2[b] + w2[j] - 2*x@w. The matmul runs in fp32r (the PE's
fast fp32 mode, RNE-rounded to 11 mantissa bits) at 4x the fp32 rate, with
full fp32-level accuracy recovered via a Dekker-style hi/lo split that
exploits the unused contraction capacity (D=64 of 128 partitions):

  mm1: lhsT=[xs_hi; xs_lo] (128 rows) rhs=[w_hi; w_hi]       -> -2x @ w_hi
  mm2: lhsT=[xs_hi; 1; 1]  (66 rows)  rhs=[w_lo; w2_hi; w2_lo]
                                              -> -2x @ w_lo + w2  (accum)
  where xs = -2x, v_hi = rne11(v), v_lo = rne11(v - v_hi).
  PSUM = -2*x'@w' + w2   with x', w' accurate to 22+ mantissa bits.
  ScalarE: out = sqrt(PSUM + x2[b])  (x2 as exact per-partition bias).

All hi/lo operands are rounded on the host (exact emulation of the HW's
fp32r RNE-11 rounding), shipped as float32r DRAM tensors.

Per core: 32 b-tiles of 128 rows; per b-tile 13 j-tiles of <=512 cols
(one PSUM bank); per b-tile a single contiguous 3.2 MB DMA store.
"""

import numpy as np
from contextlib import ExitStack

import concourse.bass as bass
import concourse.bacc as bacc
import concourse.tile as tile
from concourse import mybir
from concourse.bass_utils import run_bass_kernel_spmd

B, D, K = 4096, 64, 50000
NCORES = 8
KS = K // NCORES  # 6250 columns per core
P = 128
JT = 512          # matmul free-dim tile (one PSUM bank of fp32)
D2 = 2 * D        # 128: stacked hi/lo contraction for mm1
DL = D + 2        # 66: contraction for mm2 (w_lo + w2_hi + w2_lo rows)

F32 = mybir.dt.float32
F32R = mybir.dt.float32r


def build_nc(b=B, ks=KS):
    nbt = b // P
    nc = bacc.Bacc("TRN2", target_bir_lowering=False, debug=False)
    xs128 = nc.dram_tensor("xs128", [D2, b], F32R, kind="ExternalInput").ap()
    xs66 = nc.dram_tensor("xs66", [DL, b], F32R, kind="ExternalInput").ap()
    wst1 = nc.dram_tensor("wst1", [D2, ks], F32R, kind="ExternalInput").ap()
    wst2 = nc.dram_tensor("wst2", [DL, ks], F32R, kind="ExternalInput").ap()
    x2 = nc.dram_tensor("x2", [P, nbt], F32, kind="ExternalInput").ap()
    out = nc.dram_tensor("out", [b, ks], F32, kind="ExternalOutput").ap()

    CHUNK = 4 * JT  # 2048: one 4-bank PSUM tile, one ACT instruction
    chunks = [(c0, min(CHUNK, ks - c0)) for c0 in range(0, ks, CHUNK)]

    with tile.TileContext(nc) as tc:
        with ExitStack() as ctx:
            singles = ctx.enter_context(tc.tile_pool(name="singles", bufs=1))
            outp = ctx.enter_context(tc.tile_pool(name="outp", bufs=2))
            psum = ctx.enter_context(tc.tile_pool(name="psum", bufs=2, space="PSUM"))

            # Load order = criticality: the first j-tile's weights + x operands
            # gate the first matmuls; later weight chunks overlap with compute.
            wst1_sb = []
            wst2_sb = []
            for ic, (c0, cn) in enumerate(chunks):
                wst1_sb.append(singles.tile([D2, cn], F32R, name=f"wst1_{c0}"))
                wst2_sb.append(singles.tile([DL, cn], F32R, name=f"wst2_{c0}"))

            # chunk-0 weights and b-tile-0 x slices arrive first so the first
            # matmuls start as early as possible; the bulk follows.
            nc.sync.dma_start(out=wst1_sb[0][:, 0:JT], in_=wst1[:, 0:JT])
            xs128_sb = singles.tile([D2, b], F32R)
            nc.sync.dma_start(out=xs128_sb[:, 0:P], in_=xs128[:, 0:P])
            nc.sync.dma_start(out=wst2_sb[0][:, 0:JT], in_=wst2[:, 0:JT])
            xs66_sb = singles.tile([DL, b], F32R)
            nc.sync.dma_start(out=xs66_sb[:, 0:P], in_=xs66[:, 0:P])
            x2_sb = singles.tile([P, nbt], F32)
            nc.sync.dma_start(out=x2_sb, in_=x2)
            c0n = chunks[0][1]
            nc.sync.dma_start(out=wst1_sb[0][:, JT:c0n], in_=wst1[:, JT:c0n])
            nc.sync.dma_start(out=wst2_sb[0][:, JT:c0n], in_=wst2[:, JT:c0n])
            nc.sync.dma_start(out=xs128_sb[:, P:b], in_=xs128[:, P:b])
            nc.sync.dma_start(out=xs66_sb[:, P:b], in_=xs66[:, P:b])
            for ic, (c0, cn) in enumerate(chunks):
                if ic == 0:
                    continue
                nc.sync.dma_start(out=wst1_sb[ic], in_=wst1[:, c0:c0 + cn])
                nc.sync.dma_start(out=wst2_sb[ic], in_=wst2[:, c0:c0 + cn])

            for ib in range(nbt):
                # Store per chunk only on the first b-tile (starts the store
                # pipeline early); whole-row 3.2 MB stores otherwise — large
                # stores measurably minimize total DMA engine-seconds.
                chunked_store = ib == 0
                ot = outp.tile([P, ks], F32)
                for ic, (c0, cn) in enumerate(chunks):
                    pt = psum.tile([P, CHUNK], F32)
                    for jj in range(0, cn, JT):
                        jn = min(JT, cn - jj)
                        nc.tensor.matmul(
                            pt[:, jj:jj + jn],
                            xs128_sb[:, ib * P:(ib + 1) * P],
                            wst1_sb[ic][:, jj:jj + jn],
                            start=True,
                            stop=False,
                        )
                        nc.tensor.matmul(
                            pt[:, jj:jj + jn],
                            xs66_sb[:, ib * P:(ib + 1) * P],
                            wst2_sb[ic][:, jj:jj + jn],
                            start=False,
                            stop=True,
                        )
                    nc.scalar.activation(
                        ot[:, c0:c0 + cn],
                        pt[:, :cn],
                        mybir.ActivationFunctionType.Sqrt,
                        bias=x2_sb[:, ib:ib + 1],
                        scale=1.0,
                    )
                    if chunked_store:
                        nc.sync.dma_start(
                            out=out[ib * P:(ib + 1) * P, c0:c0 + cn],
                            in_=ot[:, c0:c0 + cn],
                        )
                if not chunked_store:
                    nc.sync.dma_start(out=out[ib * P:(ib + 1) * P, :], in_=ot)
    nc.compile()
    return nc


def _rne11(x):
    """HW-exact fp32r rounding: RNE to 11 mantissa bits."""
    x = np.asarray(x, np.float32)
    u = x.view(np.uint32).astype(np.uint64)
    shift = np.uint64(12)
    half = np.uint64(1 << 11)
    lsb = (u >> shift) & np.uint64(1)
    u2 = (u + half - np.uint64(1) + lsb) >> shift << shift
    return u2.astype(np.uint32).view(np.float32)


def prep_inputs(x, weight):
    """Host-side prep: hi/lo fp32r splits and stacked operand matrices."""
    x = np.ascontiguousarray(x, dtype=np.float32)
    weight = np.ascontiguousarray(weight, dtype=np.float32)
    b, d = x.shape
    k = weight.shape[1]
    x2 = (x.astype(np.float64) ** 2).sum(axis=1).astype(np.float32)
    w2 = (weight.astype(np.float64) ** 2).sum(axis=0).astype(np.float32)

    xs = (-2.0 * x).astype(np.float32)
    xs_hi = _rne11(xs)
    xs_lo = _rne11((xs - xs_hi).astype(np.float32))
    w_hi = _rne11(weight)
    w_lo = _rne11((weight - w_hi).astype(np.float32))
    w2_hi = _rne11(w2)
    w2_lo = _rne11((w2 - w2_hi).astype(np.float32))

    xs128 = np.empty((D2, b), dtype=np.float32)
    xs128[:d] = xs_hi.T
    xs128[d:] = xs_lo.T
    xs66 = np.empty((DL, b), dtype=np.float32)
    xs66[:d] = xs_hi.T
    xs66[d:] = 1.0
    wst1 = np.empty((D2, k), dtype=np.float32)
    wst1[:d] = w_hi
    wst1[d:] = w_hi
    wst2 = np.empty((DL, k), dtype=np.float32)
    wst2[:d] = w_lo
    wst2[d] = w2_hi
    wst2[d + 1] = w2_lo
    x2t = np.ascontiguousarray(x2.reshape(b // P, P).T)  # [P, NBT]
    return xs128, xs66, wst1, wst2, x2t


_nc_cache = {}


def _get_nc():
    if "nc" not in _nc_cache:
        _nc_cache["nc"] = build_nc()
    return _nc_cache["nc"]


def make_in_maps(x, weight, ks=KS):
    xs128, xs66, wst1, wst2, x2t = prep_inputs(x, weight)
    return [
        {"xs128": xs128,
         "xs66": xs66,
         "wst1": np.ascontiguousarray(wst1[:, i * ks:(i + 1) * ks]),
         "wst2": np.ascontiguousarray(wst2[:, i * ks:(i + 1) * ks]),
         "x2": x2t}
        for i in range(NCORES)
    ]


def kernel(x, weight):
    nc = _get_nc()
    in_maps = make_in_maps(x, weight)
    res = run_bass_kernel_spmd(nc, in_maps, core_ids=list(range(NCORES)))
    return np.concatenate([res.results[i]["out"] for i in range(NCORES)], axis=1)



# revision 2
# speedup vs baseline: 1.3711x; 1.3711x over previous
"""Euclidean distance (cdist) kernel for Trainium2, 8 NeuronCores.

out[b, j] = || x[b, :] - weight[:, j] ||_2   for x [4096, 64], weight [64, 50000].

Sharding (per hint): K = 50000 split into 8 slabs of 6250, one per core
(tensor-parallel over prototypes); x replicated; no cross-core reduction.

Math: dist^2 = x2[b] + w2[j] - 2*x@w, computed with ONE fp32r matmul of
contraction 66 (the PE's fast fp32 mode, RNE-rounded to 11 mantissa bits):

  lhsT = [-2x (64 rows); 1; 1]          [66, B]
  rhs  = [w (64 rows); w2_hi; w2_lo]    [66, K/8]
  PSUM = -2x@w + w2                     (w2 split so its 11-bit rounding
                                         error cancels to ~2^-22 rel)
  ACT:   out_fp16 = sqrt(PSUM + x2[b])  (x2 as exact per-partition bias)

The output is stored as fp16 (rel err 2^-11 ~ 5e-4, tolerance is 2e-2),
halving HBM store traffic vs fp32 - the dominant cost for this shape.
Host upcasts to fp32. Measured end-to-end max rel err ~5e-4.

Per core: 32 b-tiles of 128 rows; per b-tile 4 PSUM chunks (2048 x3 + 106)
ping-ponged across the 8 PSUM banks; ACT drains each chunk with a single
sqrt+bias instruction; one contiguous 1.6 MB fp16 DMA store per b-tile.
"""

import numpy as np
from contextlib import ExitStack

import concourse.bass as bass
import concourse.bacc as bacc
import concourse.tile as tile
from concourse import mybir
from concourse.bass_utils import run_bass_kernel_spmd

B, D, K = 4096, 64, 50000
NCORES = 8
KS = K // NCORES  # 6250 columns per core
P = 128
JT = 512          # matmul free-dim tile (one PSUM bank of fp32)
DL = D + 2        # 66: contraction rows (w + w2_hi + w2_lo)

F32 = mybir.dt.float32
F32R = mybir.dt.float32r
F16 = mybir.dt.float16


def build_nc(b=B, ks=KS):
    nbt = b // P
    nc = bacc.Bacc("TRN2", target_bir_lowering=False, debug=False)
    xs66 = nc.dram_tensor("xs66", [DL, b], F32R, kind="ExternalInput").ap()
    wst = nc.dram_tensor("wst", [DL, ks], F32R, kind="ExternalInput").ap()
    x2 = nc.dram_tensor("x2", [P, nbt], F32, kind="ExternalInput").ap()
    out = nc.dram_tensor("out", [b, ks], F16, kind="ExternalOutput").ap()

    CHUNK = 4 * JT  # 2048: one 4-bank PSUM tile, one ACT instruction
    chunks = [(c0, min(CHUNK, ks - c0)) for c0 in range(0, ks, CHUNK)]

    with tile.TileContext(nc) as tc:
        with ExitStack() as ctx:
            singles = ctx.enter_context(tc.tile_pool(name="singles", bufs=1))
            outp = ctx.enter_context(tc.tile_pool(name="outp", bufs=2))
            psum = ctx.enter_context(tc.tile_pool(name="psum", bufs=2, space="PSUM"))

            wst_sb = []
            for ic, (c0, cn) in enumerate(chunks):
                wst_sb.append(singles.tile([DL, cn], F32R, name=f"wst_{c0}"))

            # chunk-0 weights and b-tile-0 x slices arrive first so the first
            # matmuls start as early as possible; the bulk follows.
            nc.sync.dma_start(out=wst_sb[0][:, 0:JT], in_=wst[:, 0:JT])
            xs66_sb = singles.tile([DL, b], F32R)
            nc.sync.dma_start(out=xs66_sb[:, 0:P], in_=xs66[:, 0:P])
            x2_sb = singles.tile([P, nbt], F32)
            nc.sync.dma_start(out=x2_sb, in_=x2)
            c0n = chunks[0][1]
            nc.sync.dma_start(out=wst_sb[0][:, JT:c0n], in_=wst[:, JT:c0n])
            nc.sync.dma_start(out=xs66_sb[:, P:b], in_=xs66[:, P:b])
            for ic, (c0, cn) in enumerate(chunks):
                if ic == 0:
                    continue
                nc.sync.dma_start(out=wst_sb[ic], in_=wst[:, c0:c0 + cn])

            for ib in range(nbt):
                # Store per chunk only on the first b-tile (starts the store
                # pipeline early); whole-row 1.6 MB stores otherwise — large
                # stores measurably minimize total DMA engine-seconds.
                chunked_store = ib == 0
                ot = outp.tile([P, ks], F16)
                for ic, (c0, cn) in enumerate(chunks):
                    pt = psum.tile([P, cn], F32)
                    for jj in range(0, cn, JT):
                        jn = min(JT, cn - jj)
                        nc.tensor.matmul(
                            pt[:, jj:jj + jn],
                            xs66_sb[:, ib * P:(ib + 1) * P],
                            wst_sb[ic][:, jj:jj + jn],
                            start=True,
                            stop=True,
                        )
                    nc.scalar.activation(
                        ot[:, c0:c0 + cn],
                        pt[:, :cn],
                        mybir.ActivationFunctionType.Sqrt,
                        bias=x2_sb[:, ib:ib + 1],
                        scale=1.0,
                    )
                    if chunked_store:
                        nc.sync.dma_start(
                            out=out[ib * P:(ib + 1) * P, c0:c0 + cn],
                            in_=ot[:, c0:c0 + cn],
                        )
                if not chunked_store:
                    nc.sync.dma_start(out=out[ib * P:(ib + 1) * P, :], in_=ot)
    nc.compile()
    return nc


def _rne11(x):
    """HW-exact fp32r rounding: RNE to 11 mantissa bits."""
    x = np.asarray(x, np.float32)
    u = x.view(np.uint32).astype(np.uint64)
    shift = np.uint64(12)
    half = np.uint64(1 << 11)
    lsb = (u >> shift) & np.uint64(1)
    u2 = (u + half - np.uint64(1) + lsb) >> shift << shift
    return u2.astype(np.uint32).view(np.float32)


def prep_inputs(x, weight):
    """Host-side prep: stacked [66, .] operand matrices and exact x2."""
    x = np.ascontiguousarray(x, dtype=np.float32)
    weight = np.ascontiguousarray(weight, dtype=np.float32)
    b, d = x.shape
    k = weight.shape[1]
    x2 = (x.astype(np.float64) ** 2).sum(axis=1).astype(np.float32)
    w2 = (weight.astype(np.float64) ** 2).sum(axis=0).astype(np.float32)

    w2_hi = _rne11(w2)
    w2_lo = (w2 - w2_hi).astype(np.float32)

    xs66 = np.empty((DL, b), dtype=np.float32)
    xs66[:d] = (-2.0 * x).astype(np.float32).T
    xs66[d:] = 1.0
    wst = np.empty((DL, k), dtype=np.float32)
    wst[:d] = weight
    wst[d] = w2_hi
    wst[d + 1] = w2_lo
    x2t = np.ascontiguousarray(x2.reshape(b // P, P).T)  # [P, NBT]
    return xs66, wst, x2t


_nc_cache = {}


def _get_nc():
    if "nc" not in _nc_cache:
        _nc_cache["nc"] = build_nc()
    return _nc_cache["nc"]


def make_in_maps(x, weight, ks=KS):
    xs66, wst, x2t = prep_inputs(x, weight)
    return [
        {"xs66": xs66,
         "wst": np.ascontiguousarray(wst[:, i * ks:(i + 1) * ks]),
         "x2": x2t}
        for i in range(NCORES)
    ]


def kernel(x, weight):
    nc = _get_nc()
    in_maps = make_in_maps(x, weight)
    res = run_bass_kernel_spmd(nc, in_maps, core_ids=list(range(NCORES)))
    out = np.concatenate([res.results[i]["out"] for i in range(NCORES)], axis=1)
    return out.astype(np.float32)


# revision 4
# speedup vs baseline: 1.4741x; 1.0751x over previous
"""Euclidean distance (cdist) kernel for Trainium2, 8 NeuronCores.

out[b, j] = || x[b, :] - weight[:, j] ||_2   for x [4096, 64], weight [64, 50000].

Sharding (per hint): K = 50000 split into 8 slabs of 6250, one per core
(tensor-parallel over prototypes); x replicated; no cross-core reduction.

Math: dist^2 = x2[b] + w2[j] - 2*x@w, computed with ONE bf16 matmul of
contraction 66 (bf16 runs the PE at full rate with fast weight loads,
vs the 3x-slower fp32 path):

  lhsT = [-2x (64 rows); 1; 1]          [66, B]    bf16
  rhs  = [w (64 rows); w2_hi; w2_lo]    [66, K/8]  bf16
  PSUM = -2x@w + w2   (fp32 accum; w2 split hi/lo so its bf16 rounding
                       error drops to ~2^-17 rel)
  ACT:   out_fp16 = sqrt(PSUM + x2[b])  (x2 as exact fp32 per-partition bias)

The output is stored as fp16 (rel err ~5e-4 incl bf16 matmul rounding;
tolerance is 2e-2), halving HBM store traffic vs fp32 - the dominant
memory cost for this shape. Host upcasts to fp32.

Per core: 32 b-tiles of 128 rows; per b-tile 4 PSUM chunks (2048 x3 + 106)
ping-ponged across the 8 PSUM banks; ACT drains each chunk with a single
sqrt+bias instruction (the critical engine: ~1 elem/cycle/lane @ 1.2 GHz);
one contiguous 1.6 MB fp16 DMA store per b-tile, triple-buffered.
"""

import numpy as np
import ml_dtypes
from contextlib import ExitStack

import concourse.bass as bass
import concourse.bacc as bacc
import concourse.tile as tile
from concourse import mybir
from concourse.bass_utils import run_bass_kernel_spmd

B, D, K = 4096, 64, 50000
NCORES = 8
KS = K // NCORES  # 6250 columns per core
P = 128
JT = 512          # matmul free-dim tile (one PSUM bank of fp32 output)
DL = D + 2        # 66: contraction rows (w + w2_hi + w2_lo)

F32 = mybir.dt.float32
BF16 = mybir.dt.bfloat16
F16 = mybir.dt.float16
NP_BF16 = ml_dtypes.bfloat16


def build_nc(b=B, ks=KS):
    nbt = b // P
    nc = bacc.Bacc("TRN2", target_bir_lowering=False, debug=False)
    xs66 = nc.dram_tensor("xs66", [DL, b], BF16, kind="ExternalInput").ap()
    wst = nc.dram_tensor("wst", [DL, ks], BF16, kind="ExternalInput").ap()
    x2 = nc.dram_tensor("x2", [P, nbt], F32, kind="ExternalInput").ap()
    out = nc.dram_tensor("out", [b, ks], F16, kind="ExternalOutput").ap()

    CHUNK = 2048    # one 4-bank PSUM tile, one ACT instruction
    chunks = [(c0, min(CHUNK, ks - c0)) for c0 in range(0, ks, CHUNK)]

    with tile.TileContext(nc) as tc:
        with ExitStack() as ctx:
            singles = ctx.enter_context(tc.tile_pool(name="singles", bufs=1))
            outp = ctx.enter_context(tc.tile_pool(name="outp", bufs=3))
            psum = ctx.enter_context(tc.tile_pool(name="psum", bufs=2, space="PSUM"))

            wst_sb = []
            for ic, (c0, cn) in enumerate(chunks):
                wst_sb.append(singles.tile([DL, cn], BF16, name=f"wst_{c0}"))

            # chunk-0 weights and b-tile-0 x slices arrive first so the first
            # matmuls start as early as possible; the bulk follows.
            nc.sync.dma_start(out=wst_sb[0], in_=wst[:, 0:CHUNK])
            xs66_sb = singles.tile([DL, b], BF16)
            nc.sync.dma_start(out=xs66_sb[:, 0:P], in_=xs66[:, 0:P])
            x2_sb = singles.tile([P, nbt], F32)
            nc.sync.dma_start(out=x2_sb, in_=x2)
            nc.sync.dma_start(out=xs66_sb[:, P:b], in_=xs66[:, P:b])
            for ic, (c0, cn) in enumerate(chunks):
                if ic == 0:
                    continue
                nc.sync.dma_start(out=wst_sb[ic], in_=wst[:, c0:c0 + cn])

            for ib in range(nbt):
                # Store per chunk only on the first b-tile (starts the store
                # pipeline early); whole-row 1.6 MB stores otherwise — large
                # stores measurably minimize total DMA engine-seconds.
                chunked_store = ib == 0
                ot = outp.tile([P, ks], F16)
                for ic, (c0, cn) in enumerate(chunks):
                    pt = psum.tile([P, cn], F32)
                    for jj in range(0, cn, JT):
                        jn = min(JT, cn - jj)
                        nc.tensor.matmul(
                            pt[:, jj:jj + jn],
                            xs66_sb[:, ib * P:(ib + 1) * P],
                            wst_sb[ic][:, jj:jj + jn],
                            start=True,
                            stop=True,
                        )
                    nc.scalar.activation(
                        ot[:, c0:c0 + cn],
                        pt[:, :cn],
                        mybir.ActivationFunctionType.Sqrt,
                        bias=x2_sb[:, ib:ib + 1],
                        scale=1.0,
                    )
                    if chunked_store:
                        nc.sync.dma_start(
                            out=out[ib * P:(ib + 1) * P, c0:c0 + cn],
                            in_=ot[:, c0:c0 + cn],
                        )
                if not chunked_store:
                    nc.sync.dma_start(out=out[ib * P:(ib + 1) * P, :], in_=ot)
    nc.compile()
    return nc


def prep_inputs(x, weight):
    """Host-side prep: stacked bf16 [66, .] operand matrices and exact x2."""
    x = np.ascontiguousarray(x, dtype=np.float32)
    weight = np.ascontiguousarray(weight, dtype=np.float32)
    b, d = x.shape
    k = weight.shape[1]
    x2 = (x.astype(np.float64) ** 2).sum(axis=1).astype(np.float32)
    w2 = (weight.astype(np.float64) ** 2).sum(axis=0).astype(np.float32)

    w2_hi = w2.astype(NP_BF16).astype(np.float32)
    w2_lo = w2 - w2_hi

    xs66 = np.empty((DL, b), dtype=NP_BF16)
    xs66[:d] = (-2.0 * x).T.astype(NP_BF16)
    xs66[d:] = 1.0
    wst = np.empty((DL, k), dtype=NP_BF16)
    wst[:d] = weight.astype(NP_BF16)
    wst[d] = w2_hi.astype(NP_BF16)
    wst[d + 1] = w2_lo.astype(NP_BF16)
    x2t = np.ascontiguousarray(x2.reshape(b // P, P).T)  # [P, NBT]
    return xs66, wst, x2t


_nc_cache = {}


def _get_nc():
    if "nc" not in _nc_cache:
        _nc_cache["nc"] = build_nc()
    return _nc_cache["nc"]


def make_in_maps(x, weight, ks=KS):
    xs66, wst, x2t = prep_inputs(x, weight)
    return [
        {"xs66": xs66,
         "wst": np.ascontiguousarray(wst[:, i * ks:(i + 1) * ks]),
         "x2": x2t}
        for i in range(NCORES)
    ]


def kernel(x, weight):
    nc = _get_nc()
    in_maps = make_in_maps(x, weight)
    res = run_bass_kernel_spmd(nc, in_maps, core_ids=list(range(NCORES)))
    out = np.concatenate([res.results[i]["out"] for i in range(NCORES)], axis=1)
    return out.astype(np.float32)


# revision 7
# speedup vs baseline: 1.7356x; 1.1774x over previous
"""Euclidean distance (cdist) kernel for Trainium2, 8 NeuronCores.

out[b, j] = || x[b, :] - weight[:, j] ||_2   for x [4096, 64], weight [64, 50000].

Sharding (per hint): K = 50000 split into 8 slabs of 6250, one per core
(tensor-parallel over prototypes); x replicated; no cross-core reduction.

Math: dist^2 = x2[b] + w2[j] - 2*x@w, computed with ONE bf16 matmul of
contraction 68 (bf16 runs the PE at full rate with fast weight loads):

  lhsT = [-2x (64); 1; 1; x2_hi; x2_lo]   [68, B]    bf16
  rhs  = [w (64); w2_hi; w2_lo; 1; 1]     [68, K/8]  bf16
  PSUM = -2x@w + w2 + x2  (fp32 accum; w2/x2 split hi/lo so their bf16
                           rounding error drops to ~2^-16 rel)
  ACT:   out_fp16 = sqrt(PSUM)

The output is stored as fp16 (rel err ~2e-3 incl bf16 matmul rounding;
tolerance is 2e-2), halving HBM store traffic vs fp32 - the dominant
memory cost for this shape. Host upcasts to fp32.

The critical engine is ScalarE (sqrt: 1 elem/cycle/lane @ 1.2 GHz =
167 us/core floor + per-instruction PSUM-access init). Structure keeps
it saturated:
 - chunk-major loop: for each 2048-col chunk column, sweep all 32
   b-tiles; PSUM ping-pongs 2x4 banks; every main ACT is a uniform
   FD=2048 instruction whose PE refill is fully covered by the
   previous ACT (no b-tile-boundary stall of the row-major order).
 - the K-tail (6250 - 3*2048 = 106 cols) is computed TRANSPOSED
   (lhsT=w_tail, moving=xs -> psum [106, 2048]) so it costs 2 big ACT
   instructions instead of 32 tiny ones; host transposes it back.
 - folding x2 into the matmul (vs an ACT per-b-tile bias) is what
   makes tail ACTs b-tile-agnostic.
"""

import numpy as np
import ml_dtypes
from contextlib import ExitStack

import concourse.bass as bass
import concourse.bacc as bacc
import concourse.tile as tile
from concourse import mybir
from concourse.bass_utils import run_bass_kernel_spmd

B, D, K = 4096, 64, 50000
NCORES = 8
KS = K // NCORES   # 6250 columns per core
P = 128
JT = 512           # matmul free-dim tile (one PSUM bank of fp32 output)
DL = D + 4         # 68 contraction rows: w | w2_hi | w2_lo | 1 | 1
CHUNK = 2048       # one 4-bank PSUM tile, one ACT instruction
NMAIN = KS // CHUNK            # 3 full chunk columns
KMAIN = NMAIN * CHUNK          # 6144
KT = KS - KMAIN                # 106 tail columns

F32 = mybir.dt.float32
BF16 = mybir.dt.bfloat16
F16 = mybir.dt.float16
NP_BF16 = ml_dtypes.bfloat16


def build_nc(b=B):
    nbt = b // P
    nc = bacc.Bacc("TRN2", target_bir_lowering=False, debug=False)
    xs68 = nc.dram_tensor("xs68", [DL, b], BF16, kind="ExternalInput").ap()
    wst = nc.dram_tensor("wst", [DL, KS], BF16, kind="ExternalInput").ap()
    out = nc.dram_tensor("out", [b, KMAIN], F16, kind="ExternalOutput").ap()
    out_t = nc.dram_tensor("out_t", [KT, b], F16, kind="ExternalOutput").ap()

    with tile.TileContext(nc) as tc:
        with ExitStack() as ctx:
            singles = ctx.enter_context(tc.tile_pool(name="singles", bufs=1))
            outp = ctx.enter_context(tc.tile_pool(name="outp", bufs=4))
            psum = ctx.enter_context(tc.tile_pool(name="psum", bufs=2, space="PSUM"))

            wst_sb = [singles.tile([DL, CHUNK], BF16, name=f"wst_{c}")
                      for c in range(NMAIN)]
            wtail_sb = singles.tile([DL, KT], BF16)
            xs68_sb = singles.tile([DL, b], BF16)

            # Load order = criticality: chunk-0's first matmul tile and the
            # first x b-tile gate the pipeline start; the bulk follows.
            nc.sync.dma_start(out=wst_sb[0][:, 0:JT], in_=wst[:, 0:JT])
            nc.sync.dma_start(out=xs68_sb[:, 0:P], in_=xs68[:, 0:P])
            nc.sync.dma_start(out=wst_sb[0][:, JT:CHUNK], in_=wst[:, JT:CHUNK])
            nc.sync.dma_start(out=xs68_sb[:, P:b], in_=xs68[:, P:b])
            for c in range(1, NMAIN):
                nc.sync.dma_start(out=wst_sb[c], in_=wst[:, c * CHUNK:(c + 1) * CHUNK])
            nc.sync.dma_start(out=wtail_sb, in_=wst[:, KMAIN:KS])

            # Main phase: chunk-major. Every ACT is FD=2048 from a 4-bank
            # PSUM tile; PE refills one tile while ACT drains the other.
            for c in range(NMAIN):
                for ib in range(nbt):
                    pt = psum.tile([P, CHUNK], F32, name="pt", tag="pt")
                    for jj in range(0, CHUNK, JT):
                        nc.tensor.matmul(
                            pt[:, jj:jj + JT],
                            xs68_sb[:, ib * P:(ib + 1) * P],
                            wst_sb[c][:, jj:jj + JT],
                            start=True,
                            stop=True,
                        )
                    ot = outp.tile([P, CHUNK], F16, name="ot")
                    nc.scalar.activation(
                        ot,
                        pt,
                        mybir.ActivationFunctionType.Sqrt,
                        scale=1.0,
                    )
                    nc.sync.dma_start(
                        out=out[ib * P:(ib + 1) * P, c * CHUNK:(c + 1) * CHUNK],
                        in_=ot,
                    )

            # Tail phase, transposed: psum[j, b] = sum_c wst[c, j] * xs[c, b]
            # (the stacked operands make this the same dist^2 by symmetry).
            ot_t = outp.tile([KT, b], F16, tag="tailout", bufs=1)
            for h in range(b // CHUNK):
                pt = psum.tile([P, CHUNK], F32, name="pt_t", tag="pt")
                for jj in range(0, CHUNK, JT):
                    nc.tensor.matmul(
                        pt[0:KT, jj:jj + JT],
                        wtail_sb,
                        xs68_sb[:, h * CHUNK + jj:h * CHUNK + jj + JT],
                        start=True,
                        stop=True,
                    )
                nc.scalar.activation(
                    ot_t[:, h * CHUNK:(h + 1) * CHUNK],
                    pt[0:KT, :],
                    mybir.ActivationFunctionType.Sqrt,
                    scale=1.0,
                )
            nc.sync.dma_start(out=out_t, in_=ot_t)
    nc.compile()
    return nc


def _split_bf16(v):
    hi = v.astype(NP_BF16).astype(np.float32)
    lo = (v - hi).astype(NP_BF16)
    return hi.astype(NP_BF16), lo


def prep_inputs(x, weight):
    """Host-side prep: stacked bf16 [68, .] operand matrices."""
    x = np.ascontiguousarray(x, dtype=np.float32)
    weight = np.ascontiguousarray(weight, dtype=np.float32)
    b, d = x.shape
    k = weight.shape[1]
    x2 = (x.astype(np.float64) ** 2).sum(axis=1).astype(np.float32)
    w2 = (weight.astype(np.float64) ** 2).sum(axis=0).astype(np.float32)
    w2_hi, w2_lo = _split_bf16(w2)
    x2_hi, x2_lo = _split_bf16(x2)

    xs68 = np.empty((DL, b), dtype=NP_BF16)
    xs68[:d] = (-2.0 * x).T.astype(NP_BF16)
    xs68[d] = 1.0
    xs68[d + 1] = 1.0
    xs68[d + 2] = x2_hi
    xs68[d + 3] = x2_lo
    wst = np.empty((DL, k), dtype=NP_BF16)
    wst[:d] = weight.astype(NP_BF16)
    wst[d] = w2_hi
    wst[d + 1] = w2_lo
    wst[d + 2] = 1.0
    wst[d + 3] = 1.0
    return xs68, wst


_nc_cache = {}


def _get_nc():
    if "nc" not in _nc_cache:
        _nc_cache["nc"] = build_nc()
    return _nc_cache["nc"]


def make_in_maps(x, weight, ks=KS):
    xs68, wst = prep_inputs(x, weight)
    return [
        {"xs68": xs68,
         "wst": np.ascontiguousarray(wst[:, i * ks:(i + 1) * ks])}
        for i in range(NCORES)
    ]


def kernel(x, weight):
    nc = _get_nc()
    in_maps = make_in_maps(x, weight)
    res = run_bass_kernel_spmd(nc, in_maps, core_ids=list(range(NCORES)))
    parts = []
    for i in range(NCORES):
        main = res.results[i]["out"]          # [B, KMAIN] fp16
        tail = res.results[i]["out_t"]        # [KT, B] fp16
        parts.append(np.concatenate([main, tail.T], axis=1))
    return np.concatenate(parts, axis=1).astype(np.float32)


# revision 9
# speedup vs baseline: 1.7601x; 1.0141x over previous
"""Euclidean distance (cdist) kernel for Trainium2, 8 NeuronCores.

out[b, j] = || x[b, :] - weight[:, j] ||_2   for x [4096, 64], weight [64, 50000].

Sharding (per hint): K = 50000 split into 8 slabs of 6250, one per core
(tensor-parallel over prototypes); x replicated; no cross-core reduction.

Math: dist^2 = x2[b] + w2[j] - 2*x@w, computed with ONE bf16 matmul of
contraction 68 (bf16 runs the PE at full rate with fast weight loads):

  lhsT = [-2x (64); 1; 1; x2_hi; x2_lo]   [68, B]    bf16
  rhs  = [w (64); w2_hi; w2_lo; 1; 1]     [68, K/8]  bf16
  PSUM = -2x@w + w2 + x2  (fp32 accum; w2/x2 split hi/lo so their bf16
                           rounding error drops to ~2^-16 rel)
  ACT:   out_fp16 = sqrt(PSUM)

The output is stored as fp16 (rel err ~2e-3 incl bf16 matmul rounding;
tolerance is 2e-2), halving HBM store traffic vs fp32 - the dominant
memory cost for this shape. Host upcasts to fp32.

The critical engine is ScalarE (sqrt: 1 elem/cycle/lane @ 1.2 GHz =
167 us/core floor + per-instruction PSUM-access init). Structure keeps
it saturated:
 - chunk-major loop: for each 2048-col chunk column, sweep all 32
   b-tiles; PSUM ping-pongs 2x4 banks; every main ACT is a uniform
   FD=2048 instruction whose PE refill is fully covered by the
   previous ACT (no b-tile-boundary stall of the row-major order).
 - the K-tail (6250 - 3*2048 = 106 cols) is computed TRANSPOSED
   (lhsT=w_tail, moving=xs -> psum [106, 2048]) so it costs 2 big ACT
   instructions instead of 32 tiny ones; host transposes it back.
 - folding x2 into the matmul (vs an ACT per-b-tile bias) is what
   makes tail ACTs b-tile-agnostic.
"""

import numpy as np
import ml_dtypes
from contextlib import ExitStack

import concourse.bass as bass
import concourse.bacc as bacc
import concourse.tile as tile
from concourse import mybir
from concourse.bass_utils import run_bass_kernel_spmd

B, D, K = 4096, 64, 50000
NCORES = 8
KS = K // NCORES   # 6250 columns per core
P = 128
JT = 512           # matmul free-dim tile (one PSUM bank of fp32 output)
DL = D + 4         # 68 contraction rows: w | w2_hi | w2_lo | 1 | 1
CHUNK = 2048       # one 4-bank PSUM tile, one ACT instruction
NMAIN = KS // CHUNK            # 3 full chunk columns
KMAIN = NMAIN * CHUNK          # 6144
KT = KS - KMAIN                # 106 tail columns

F32 = mybir.dt.float32
BF16 = mybir.dt.bfloat16
F16 = mybir.dt.float16
NP_BF16 = ml_dtypes.bfloat16


def build_nc(b=B):
    nbt = b // P
    nc = bacc.Bacc("TRN2", target_bir_lowering=False, debug=False)
    xs68 = nc.dram_tensor("xs68", [DL, b], BF16, kind="ExternalInput").ap()
    wst = nc.dram_tensor("wst", [DL, KS], BF16, kind="ExternalInput").ap()
    out = nc.dram_tensor("out", [b, KMAIN], F16, kind="ExternalOutput").ap()
    out_t = nc.dram_tensor("out_t", [KT, b], F16, kind="ExternalOutput").ap()

    with tile.TileContext(nc) as tc:
        with ExitStack() as ctx:
            singles = ctx.enter_context(tc.tile_pool(name="singles", bufs=1))
            outp = ctx.enter_context(tc.tile_pool(name="outp", bufs=4))
            psum = ctx.enter_context(tc.tile_pool(name="psum", bufs=2, space="PSUM"))

            wst_sb = [singles.tile([DL, CHUNK], BF16, name=f"wst_{c}")
                      for c in range(NMAIN)]
            wtail_sb = singles.tile([DL, KT], BF16)
            xs68_sb = singles.tile([DL, b], BF16)

            # Load order = criticality: the first x b-tile and chunk-0's
            # weights gate the pipeline start. Chunk-0 is loaded in 512-col
            # pieces so each matmul's operand lands (and its completion
            # semaphore fires) as early as possible; the bulk follows.
            nc.sync.dma_start(out=xs68_sb[:, 0:P], in_=xs68[:, 0:P])
            for jj in range(0, CHUNK, JT):
                nc.sync.dma_start(out=wst_sb[0][:, jj:jj + JT], in_=wst[:, jj:jj + JT])
            nc.sync.dma_start(out=xs68_sb[:, P:b], in_=xs68[:, P:b])
            nc.sync.dma_start(out=wtail_sb, in_=wst[:, KMAIN:KS])
            for c in range(1, NMAIN):
                nc.sync.dma_start(out=wst_sb[c], in_=wst[:, c * CHUNK:(c + 1) * CHUNK])

            # Main phase: chunk-major. Every ACT is FD=2048 from a 4-bank
            # PSUM tile; PE refills one tile while ACT drains the other.
            # The K-tail runs between columns 0 and 1 so its (slow, small-
            # descriptor) stores overlap with main compute instead of
            # serializing at the kernel end.
            def main_column(c):
                for ib in range(nbt):
                    pt = psum.tile([P, CHUNK], F32, name="pt", tag="pt")
                    for jj in range(0, CHUNK, JT):
                        nc.tensor.matmul(
                            pt[:, jj:jj + JT],
                            xs68_sb[:, ib * P:(ib + 1) * P],
                            wst_sb[c][:, jj:jj + JT],
                            start=True,
                            stop=True,
                        )
                    ot = outp.tile([P, CHUNK], F16, name="ot", tag="ot")
                    nc.scalar.activation(
                        ot,
                        pt,
                        mybir.ActivationFunctionType.Sqrt,
                        scale=1.0,
                    )
                    nc.sync.dma_start(
                        out=out[ib * P:(ib + 1) * P, c * CHUNK:(c + 1) * CHUNK],
                        in_=ot,
                    )

            def tail_phase():
                # Transposed: psum[j, b] = sum_c wst[c, j] * xs[c, b]
                # (the stacked operands make this the same dist^2 by symmetry).
                for h in range(b // CHUNK):
                    pt = psum.tile([P, CHUNK], F32, name="pt_t", tag="pt")
                    for jj in range(0, CHUNK, JT):
                        nc.tensor.matmul(
                            pt[0:KT, jj:jj + JT],
                            wtail_sb,
                            xs68_sb[:, h * CHUNK + jj:h * CHUNK + jj + JT],
                            start=True,
                            stop=True,
                        )
                    ot_t = outp.tile([KT, CHUNK], F16, name="ot_t", tag="ot")
                    nc.scalar.activation(
                        ot_t,
                        pt[0:KT, :],
                        mybir.ActivationFunctionType.Sqrt,
                        scale=1.0,
                    )
                    nc.sync.dma_start(
                        out=out_t[:, h * CHUNK:(h + 1) * CHUNK],
                        in_=ot_t,
                    )

            main_column(0)
            tail_phase()
            for c in range(1, NMAIN):
                main_column(c)
    nc.compile()
    return nc


def _split_bf16(v):
    hi = v.astype(NP_BF16).astype(np.float32)
    lo = (v - hi).astype(NP_BF16)
    return hi.astype(NP_BF16), lo


def prep_inputs(x, weight):
    """Host-side prep: stacked bf16 [68, .] operand matrices."""
    x = np.ascontiguousarray(x, dtype=np.float32)
    weight = np.ascontiguousarray(weight, dtype=np.float32)
    b, d = x.shape
    k = weight.shape[1]
    x2 = (x.astype(np.float64) ** 2).sum(axis=1).astype(np.float32)
    w2 = (weight.astype(np.float64) ** 2).sum(axis=0).astype(np.float32)
    w2_hi, w2_lo = _split_bf16(w2)
    x2_hi, x2_lo = _split_bf16(x2)

    xs68 = np.empty((DL, b), dtype=NP_BF16)
    xs68[:d] = (-2.0 * x).T.astype(NP_BF16)
    xs68[d] = 1.0
    xs68[d + 1] = 1.0
    xs68[d + 2] = x2_hi
    xs68[d + 3] = x2_lo
    wst = np.empty((DL, k), dtype=NP_BF16)
    wst[:d] = weight.astype(NP_BF16)
    wst[d] = w2_hi
    wst[d + 1] = w2_lo
    wst[d + 2] = 1.0
    wst[d + 3] = 1.0
    return xs68, wst


_nc_cache = {}


def _get_nc():
    if "nc" not in _nc_cache:
        _nc_cache["nc"] = build_nc()
    return _nc_cache["nc"]


def make_in_maps(x, weight, ks=KS):
    xs68, wst = prep_inputs(x, weight)
    return [
        {"xs68": xs68,
         "wst": np.ascontiguousarray(wst[:, i * ks:(i + 1) * ks])}
        for i in range(NCORES)
    ]


def kernel(x, weight):
    nc = _get_nc()
    in_maps = make_in_maps(x, weight)
    res = run_bass_kernel_spmd(nc, in_maps, core_ids=list(range(NCORES)))
    parts = []
    for i in range(NCORES):
        main = res.results[i]["out"]          # [B, KMAIN] fp16
        tail = res.results[i]["out_t"]        # [KT, B] fp16
        parts.append(np.concatenate([main, tail.T], axis=1))
    return np.concatenate(parts, axis=1).astype(np.float32)


# revision 11
# speedup vs baseline: 1.7820x; 1.0124x over previous
"""Euclidean distance (cdist) kernel for Trainium2, 8 NeuronCores.

out[b, j] = || x[b, :] - weight[:, j] ||_2   for x [4096, 64], weight [64, 50000].

Sharding (per hint): K = 50000 split into 8 slabs of 6250, one per core
(tensor-parallel over prototypes); x replicated; no cross-core reduction.

Math: dist^2 = x2[b] + w2[j] - 2*x@w, computed with ONE bf16 matmul of
contraction 68 (bf16 runs the PE at full rate with fast weight loads):

  lhsT = [-2x (64); 1; 1; x2_hi; x2_lo]   [68, B]    bf16
  rhs  = [w (64); w2_hi; w2_lo; 1; 1]     [68, K/8]  bf16
  PSUM = -2x@w + w2 + x2  (fp32 accum; w2/x2 split hi/lo so their bf16
                           rounding error drops to ~2^-16 rel)
  ACT:   out_fp16 = sqrt(PSUM)

The output is stored as fp16 (rel err ~2e-3 incl bf16 matmul rounding;
tolerance is 2e-2), halving HBM store traffic vs fp32 - the dominant
memory cost for this shape. Host upcasts to fp32.

The critical engine is ScalarE (sqrt: 1 elem/cycle/lane @ 1.2 GHz =
167 us/core floor + per-instruction PSUM-access init). Structure keeps
it saturated:
 - chunk-major loop: for each 2048-col chunk column, sweep all 32
   b-tiles; PSUM ping-pongs 2x4 banks; every main ACT is a uniform
   FD=2048 instruction whose PE refill is fully covered by the
   previous ACT (no b-tile-boundary stall of the row-major order).
 - the K-tail (6250 - 3*2048 = 106 cols) is computed TRANSPOSED
   (lhsT=w_tail, moving=xs -> psum [106, 2048]) so it costs 2 big ACT
   instructions instead of 32 tiny ones; host transposes it back.
 - folding x2 into the matmul (vs an ACT per-b-tile bias) is what
   makes tail ACTs b-tile-agnostic.
"""

import numpy as np
import ml_dtypes
from contextlib import ExitStack

import concourse.bass as bass
import concourse.bacc as bacc
import concourse.tile as tile
from concourse import mybir
from concourse.bass_utils import run_bass_kernel_spmd

B, D, K = 4096, 64, 50000
NCORES = 8
KS = K // NCORES   # 6250 columns per core
P = 128
JT = 512           # matmul free-dim tile (one PSUM bank of fp32 output)
DL = D + 4         # 68 contraction rows: w | w2_hi | w2_lo | 1 | 1
CHUNK = 2048       # one 4-bank PSUM tile, one ACT instruction
NMAIN = KS // CHUNK            # 3 full chunk columns
KMAIN = NMAIN * CHUNK          # 6144
KT = KS - KMAIN                # 106 tail columns

F32 = mybir.dt.float32
BF16 = mybir.dt.bfloat16
F16 = mybir.dt.float16
NP_BF16 = ml_dtypes.bfloat16


def build_nc(b=B):
    nbt = b // P
    nc = bacc.Bacc("TRN2", target_bir_lowering=False, debug=False)
    xs68 = nc.dram_tensor("xs68", [DL, b], BF16, kind="ExternalInput").ap()
    wst = nc.dram_tensor("wst", [DL, KS], BF16, kind="ExternalInput").ap()
    out = nc.dram_tensor("out", [b, KMAIN], F16, kind="ExternalOutput").ap()
    out_t = nc.dram_tensor("out_t", [KT, b], F16, kind="ExternalOutput").ap()

    with tile.TileContext(nc) as tc:
        with ExitStack() as ctx:
            singles = ctx.enter_context(tc.tile_pool(name="singles", bufs=1))
            outp = ctx.enter_context(tc.tile_pool(name="outp", bufs=4))
            psum = ctx.enter_context(tc.tile_pool(name="psum", bufs=2, space="PSUM"))

            wst_sb = [singles.tile([DL, CHUNK], BF16, name=f"wst_{c}")
                      for c in range(NMAIN)]
            wtail_sb = singles.tile([DL, KT], BF16)
            xs68_sb = singles.tile([DL, b], BF16)

            # Load order = criticality: the first x b-tile and chunk-0's
            # weights gate the pipeline start. Chunk-0 is loaded in 512-col
            # pieces so each matmul's operand lands (and its completion
            # semaphore fires) as early as possible; the bulk follows.
            nc.scalar.dma_start(out=xs68_sb[:, 0:P], in_=xs68[:, 0:P])
            for jj in range(0, CHUNK, JT):
                nc.sync.dma_start(out=wst_sb[0][:, jj:jj + JT], in_=wst[:, jj:jj + JT])
            nc.scalar.dma_start(out=xs68_sb[:, P:4 * P], in_=xs68[:, P:4 * P])
            nc.sync.dma_start(out=xs68_sb[:, 4 * P:CHUNK], in_=xs68[:, 4 * P:CHUNK])
            nc.sync.dma_start(out=xs68_sb[:, CHUNK:b], in_=xs68[:, CHUNK:b])
            nc.sync.dma_start(out=wtail_sb, in_=wst[:, KMAIN:KS])
            for c in range(1, NMAIN):
                nc.sync.dma_start(out=wst_sb[c], in_=wst[:, c * CHUNK:(c + 1) * CHUNK])

            # Main phase: chunk-major. Every ACT is FD=2048 from a 4-bank
            # PSUM tile; PE refills one tile while ACT drains the other.
            # The K-tail runs between columns 0 and 1 so its (slow, small-
            # descriptor) stores overlap with main compute instead of
            # serializing at the kernel end.
            def main_column(c):
                for ib in range(nbt):
                    pt = psum.tile([P, CHUNK], F32, name="pt", tag="pt")
                    for jj in range(0, CHUNK, JT):
                        nc.tensor.matmul(
                            pt[:, jj:jj + JT],
                            xs68_sb[:, ib * P:(ib + 1) * P],
                            wst_sb[c][:, jj:jj + JT],
                            start=True,
                            stop=True,
                        )
                    ot = outp.tile([P, CHUNK], F16, name="ot", tag="ot")
                    nc.scalar.activation(
                        ot,
                        pt,
                        mybir.ActivationFunctionType.Sqrt,
                        scale=1.0,
                    )
                    nc.sync.dma_start(
                        out=out[ib * P:(ib + 1) * P, c * CHUNK:(c + 1) * CHUNK],
                        in_=ot,
                    )

            def tail_phase():
                # Transposed: psum[j, b] = sum_c wst[c, j] * xs[c, b]
                # (the stacked operands make this the same dist^2 by symmetry).
                for h in range(b // CHUNK):
                    pt = psum.tile([P, CHUNK], F32, name="pt_t", tag="pt")
                    for jj in range(0, CHUNK, JT):
                        nc.tensor.matmul(
                            pt[0:KT, jj:jj + JT],
                            wtail_sb,
                            xs68_sb[:, h * CHUNK + jj:h * CHUNK + jj + JT],
                            start=True,
                            stop=True,
                        )
                    ot_t = outp.tile([KT, CHUNK], F16, name="ot_t", tag="tailot", bufs=2)
                    nc.scalar.activation(
                        ot_t,
                        pt[0:KT, :],
                        mybir.ActivationFunctionType.Sqrt,
                        scale=1.0,
                    )
                    nc.sync.dma_start(
                        out=out_t[:, h * CHUNK:(h + 1) * CHUNK],
                        in_=ot_t,
                    )

            main_column(0)
            tail_phase()
            for c in range(1, NMAIN):
                main_column(c)
    nc.compile()
    return nc


def _split_bf16(v):
    hi = v.astype(NP_BF16).astype(np.float32)
    lo = (v - hi).astype(NP_BF16)
    return hi.astype(NP_BF16), lo


def prep_inputs(x, weight):
    """Host-side prep: stacked bf16 [68, .] operand matrices."""
    x = np.ascontiguousarray(x, dtype=np.float32)
    weight = np.ascontiguousarray(weight, dtype=np.float32)
    b, d = x.shape
    k = weight.shape[1]
    x2 = (x.astype(np.float64) ** 2).sum(axis=1).astype(np.float32)
    w2 = (weight.astype(np.float64) ** 2).sum(axis=0).astype(np.float32)
    w2_hi, w2_lo = _split_bf16(w2)
    x2_hi, x2_lo = _split_bf16(x2)

    xs68 = np.empty((DL, b), dtype=NP_BF16)
    xs68[:d] = (-2.0 * x).T.astype(NP_BF16)
    xs68[d] = 1.0
    xs68[d + 1] = 1.0
    xs68[d + 2] = x2_hi
    xs68[d + 3] = x2_lo
    wst = np.empty((DL, k), dtype=NP_BF16)
    wst[:d] = weight.astype(NP_BF16)
    wst[d] = w2_hi
    wst[d + 1] = w2_lo
    wst[d + 2] = 1.0
    wst[d + 3] = 1.0
    return xs68, wst


_nc_cache = {}


def _get_nc():
    if "nc" not in _nc_cache:
        _nc_cache["nc"] = build_nc()
    return _nc_cache["nc"]


def make_in_maps(x, weight, ks=KS):
    xs68, wst = prep_inputs(x, weight)
    return [
        {"xs68": xs68,
         "wst": np.ascontiguousarray(wst[:, i * ks:(i + 1) * ks])}
        for i in range(NCORES)
    ]


def kernel(x, weight):
    nc = _get_nc()
    in_maps = make_in_maps(x, weight)
    res = run_bass_kernel_spmd(nc, in_maps, core_ids=list(range(NCORES)))
    parts = []
    for i in range(NCORES):
        main = res.results[i]["out"]          # [B, KMAIN] fp16
        tail = res.results[i]["out_t"]        # [KT, B] fp16
        parts.append(np.concatenate([main, tail.T], axis=1))
    return np.concatenate(parts, axis=1).astype(np.float32)


# revision 16
# speedup vs baseline: 1.8598x; 1.0436x over previous
"""Euclidean distance (cdist) kernel for Trainium2, 8 NeuronCores.

out[b, j] = || x[b, :] - weight[:, j] ||_2   for x [4096, 64], weight [64, 50000].

Sharding (per hint): K = 50000 split into 8 slabs of 6250, one per core
(tensor-parallel over prototypes); x replicated; no cross-core reduction.

Math: dist^2 = x2[b] + w2[j] - 2*x@w, computed with ONE bf16 matmul of
contraction 68 (bf16 runs the PE at full rate with fast weight loads):

  lhsT = [-2x (64); 1; 1; x2_hi; x2_lo]   [68, B]    bf16
  rhs  = [w (64); w2_hi; w2_lo; 1; 1]     [68, K/8]  bf16
  PSUM = -2x@w + w2 + x2  (fp32 accum; w2/x2 split hi/lo so their bf16
                           rounding error drops to ~2^-16 rel)
  ACT:   out_fp16 = sqrt(PSUM)

The output is stored as fp16 (rel err ~2e-3 incl bf16 matmul rounding;
tolerance is 2e-2), halving HBM store traffic vs fp32 - the dominant
memory cost for this shape. Host upcasts to fp32.

The critical engine is ScalarE (sqrt: 1 elem/cycle/lane @ 1.2 GHz =
167 us/core floor + per-instruction PSUM-access init). Structure keeps
it saturated:
 - chunk-major loop: for each 2048-col chunk column, sweep all 32
   b-tiles; PSUM ping-pongs 2x4 banks; every main ACT is a uniform
   FD=2048 instruction whose PE refill is fully covered by the
   previous ACT (no b-tile-boundary stall of the row-major order).
 - the K-tail (6250 - 3*2048 = 106 cols) is computed TRANSPOSED
   (lhsT=w_tail, moving=xs -> psum [106, 2048]) so it costs 2 big ACT
   instructions instead of 32 tiny ones; host transposes it back.
 - folding x2 into the matmul (vs an ACT per-b-tile bias) is what
   makes tail ACTs b-tile-agnostic.
"""

import numpy as np
import ml_dtypes
from contextlib import ExitStack

import concourse.bass as bass
import concourse.bacc as bacc
import concourse.tile as tile
from concourse import mybir
from concourse.bass_utils import run_bass_kernel_spmd

B, D, K = 4096, 64, 50000
NCORES = 8
KS = K // NCORES   # 6250 columns per core
P = 128
JT = 512           # matmul free-dim tile (one PSUM bank of fp32 output)
DL = D + 4         # 68 contraction rows: w | w2_hi | w2_lo | 1 | 1
CHUNK = 2048       # one 4-bank PSUM tile, one ACT instruction
NMAIN = KS // CHUNK            # 3 full chunk columns
KMAIN = NMAIN * CHUNK          # 6144
KT = KS - KMAIN                # 106 tail columns

F32 = mybir.dt.float32
BF16 = mybir.dt.bfloat16
F16 = mybir.dt.float16
NP_BF16 = ml_dtypes.bfloat16


def build_nc(b=B):
    nbt = b // P
    nc = bacc.Bacc("TRN2", target_bir_lowering=False, debug=False)
    xs68 = nc.dram_tensor("xs68", [DL, b], BF16, kind="ExternalInput").ap()
    wst = nc.dram_tensor("wst", [DL, KS], BF16, kind="ExternalInput").ap()
    out = nc.dram_tensor("out", [b, KMAIN], F16, kind="ExternalOutput").ap()
    # Tail rows padded 106 -> 128: stores with a partition count that is not
    # a multiple of 16 take a degenerate 2-SDMA-engine path (~53 GB/s); the
    # padded store spreads across all 16 engines. Host slices [:106].
    out_t = nc.dram_tensor("out_t", [P, b], F16, kind="ExternalOutput").ap()

    with tile.TileContext(nc) as tc:
        with ExitStack() as ctx:
            singles = ctx.enter_context(tc.tile_pool(name="singles", bufs=1))
            outp = ctx.enter_context(tc.tile_pool(name="outp", bufs=4))
            psum = ctx.enter_context(tc.tile_pool(name="psum", bufs=2, space="PSUM"))

            wst_sb = [singles.tile([DL, CHUNK], BF16, name=f"wst_{c}")
                      for c in range(NMAIN)]
            wtail_sb = singles.tile([DL, KT], BF16)
            xs68_sb = singles.tile([DL, b], BF16)

            # Load order = criticality: the first x b-tile and chunk-0's
            # weights gate the pipeline start. Chunk-0 is loaded in 512-col
            # pieces so each matmul's operand lands (and its completion
            # semaphore fires) as early as possible; the bulk follows.
            nc.scalar.dma_start(out=xs68_sb[:, 0:P], in_=xs68[:, 0:P])
            for jj in range(0, CHUNK, JT):
                nc.sync.dma_start(out=wst_sb[0][:, jj:jj + JT], in_=wst[:, jj:jj + JT])
            nc.scalar.dma_start(out=xs68_sb[:, P:4 * P], in_=xs68[:, P:4 * P])
            nc.sync.dma_start(out=xs68_sb[:, 4 * P:CHUNK], in_=xs68[:, 4 * P:CHUNK])
            nc.sync.dma_start(out=xs68_sb[:, CHUNK:b], in_=xs68[:, CHUNK:b])
            nc.sync.dma_start(out=wtail_sb, in_=wst[:, KMAIN:KS])
            for c in range(1, NMAIN):
                nc.sync.dma_start(out=wst_sb[c], in_=wst[:, c * CHUNK:(c + 1) * CHUNK])

            # Main phase: chunk-major. Every ACT is FD=2048 from a 4-bank
            # PSUM tile; PE refills one tile while ACT drains the other.
            # The K-tail runs between columns 0 and 1 so its (slow, small-
            # descriptor) stores overlap with main compute instead of
            # serializing at the kernel end.
            def main_piece(c, ib, j0, jn):
                pt = psum.tile([P, jn], F32, name="pt", tag="pt")
                for jj in range(0, jn, JT):
                    nc.tensor.matmul(
                        pt[:, jj:jj + JT],
                        xs68_sb[:, ib * P:(ib + 1) * P],
                        wst_sb[c][:, j0 + jj:j0 + jj + JT],
                        start=True,
                        stop=True,
                    )
                ot = outp.tile([P, jn], F16, name="ot", tag="ot")
                nc.scalar.activation(
                    ot,
                    pt,
                    mybir.ActivationFunctionType.Sqrt,
                    scale=1.0,
                )
                nc.sync.dma_start(
                    out=out[ib * P:(ib + 1) * P,
                            c * CHUNK + j0:c * CHUNK + j0 + jn],
                    in_=ot,
                )

            def main_column(c, split_first=False):
                for ib in range(nbt):
                    if ib == 0 and split_first:
                        # First tile split 512 + 1536: the first ACT fires
                        # after one small DMA and one matmul instead of four.
                        main_piece(c, ib, 0, JT)
                        main_piece(c, ib, JT, CHUNK - JT)
                    else:
                        main_piece(c, ib, 0, CHUNK)

            def tail_phase():
                # Transposed: psum[j, b] = sum_c wst[c, j] * xs[c, b]
                # (the stacked operands make this the same dist^2 by symmetry).
                for h in range(b // CHUNK):
                    pt = psum.tile([P, CHUNK], F32, name="pt_t", tag="pt")
                    for jj in range(0, CHUNK, JT):
                        nc.tensor.matmul(
                            pt[0:KT, jj:jj + JT],
                            wtail_sb,
                            xs68_sb[:, h * CHUNK + jj:h * CHUNK + jj + JT],
                            start=True,
                            stop=True,
                        )
                    ot_t = outp.tile([P, CHUNK], F16, name="ot_t", tag="tailot", bufs=2)
                    nc.scalar.activation(
                        ot_t[0:KT, :],
                        pt[0:KT, :],
                        mybir.ActivationFunctionType.Sqrt,
                        scale=1.0,
                    )
                    nc.sync.dma_start(
                        out=out_t[:, h * CHUNK:(h + 1) * CHUNK],
                        in_=ot_t,
                    )

            main_column(0, split_first=True)
            tail_phase()
            for c in range(1, NMAIN):
                main_column(c)
    nc.compile()
    return nc


def _split_bf16(v):
    hi = v.astype(NP_BF16).astype(np.float32)
    lo = (v - hi).astype(NP_BF16)
    return hi.astype(NP_BF16), lo


def prep_inputs(x, weight):
    """Host-side prep: stacked bf16 [68, .] operand matrices."""
    x = np.ascontiguousarray(x, dtype=np.float32)
    weight = np.ascontiguousarray(weight, dtype=np.float32)
    b, d = x.shape
    k = weight.shape[1]
    x2 = (x.astype(np.float64) ** 2).sum(axis=1).astype(np.float32)
    w2 = (weight.astype(np.float64) ** 2).sum(axis=0).astype(np.float32)
    w2_hi, w2_lo = _split_bf16(w2)
    x2_hi, x2_lo = _split_bf16(x2)

    xs68 = np.empty((DL, b), dtype=NP_BF16)
    xs68[:d] = (-2.0 * x).T.astype(NP_BF16)
    xs68[d] = 1.0
    xs68[d + 1] = 1.0
    xs68[d + 2] = x2_hi
    xs68[d + 3] = x2_lo
    wst = np.empty((DL, k), dtype=NP_BF16)
    wst[:d] = weight.astype(NP_BF16)
    wst[d] = w2_hi
    wst[d + 1] = w2_lo
    wst[d + 2] = 1.0
    wst[d + 3] = 1.0
    return xs68, wst


_nc_cache = {}


def _get_nc():
    if "nc" not in _nc_cache:
        _nc_cache["nc"] = build_nc()
    return _nc_cache["nc"]


def make_in_maps(x, weight, ks=KS):
    xs68, wst = prep_inputs(x, weight)
    return [
        {"xs68": xs68,
         "wst": np.ascontiguousarray(wst[:, i * ks:(i + 1) * ks])}
        for i in range(NCORES)
    ]


def kernel(x, weight):
    nc = _get_nc()
    in_maps = make_in_maps(x, weight)
    res = run_bass_kernel_spmd(nc, in_maps, core_ids=list(range(NCORES)))
    parts = []
    for i in range(NCORES):
        main = res.results[i]["out"]          # [B, KMAIN] fp16
        tail = res.results[i]["out_t"][:KT]   # [P, B] fp16, rows >= KT are pad
        parts.append(np.concatenate([main, tail.T], axis=1))
    return np.concatenate(parts, axis=1).astype(np.float32)
